# revision 83
# baseline (speedup 1.0000x reference)
# HEPOS cross-attention (strided per-head K/V) on 8 Trainium2 NeuronCores.
#
# Reference computation (per head h, stride s = STRIDE_LIST[h]):
#   Q = x @ Wq.T + bq ; K = e @ Wk.T + bk ; V = e @ Wv.T + bv
#   out_h = softmax(Q_h @ K_h[::s].T / 8) @ V_h[::s]
#   out   = concat_h(out_h) @ Wo.T + bo
#
# Sharding: one (batch, 8-head half) per core. Core c owns batch c//2 and,
# for j = c%2, for each stride class ci (strides 1,2,4,8) the PAIR of heads
# (ci+8j, ci+8j+4) — both with the same stride, so per-core work is
# identical across cores. The host pre-gathers the strided encoder rows per
# class (eT1/eT2/eT4/eT8, transposed, bf16), so every device matmul has
# contiguous operands with full 128-wide contraction and output:
#   - Q/K/V projections and out-proj are pair-packed (M=128, K=128).
#   - scores are computed transposed ([S_chunk, T], K=hd=64) so the AV
#     matmul needs no transposes; the softmax denominator falls out of a
#     ones-column interleaved into the V tile.
#   - AV accumulates across all S-chunks of a class directly in PSUM.
# The 1/sqrt(hd)=1/8 score scale is folded into Wq/bq on the host.
# Phase-A work (projections) of later stride classes is interleaved into
# the attention phase of earlier classes as PE filler so the Tensor engine
# never idles (keeps the HAM clock gate at 8/8 = 2.4 GHz).
# Host sums the two per-batch partials and adds bo.

import os
import sys
from contextlib import ExitStack

import ml_dtypes
import numpy as np

BF16 = ml_dtypes.bfloat16

for _p in ("/opt/trn_rl_repo", "/root/.axon_site/_ro/trn_rl_repo"):
    if os.path.isdir(_p) and _p not in sys.path:
        sys.path.insert(0, _p)

import concourse.bass as bass  # noqa: E402
import concourse.tile as tile  # noqa: E402
from concourse import bacc, mybir  # noqa: E402
from concourse import bass_utils  # noqa: E402

F32 = mybir.dt.float32
BF = mybir.dt.bfloat16
F8 = mybir.dt.float8e4
AF = mybir.ActivationFunctionType

D_MODEL = 1024
NUM_HEADS = 16
HEAD_DIM = 64
B, T, S = 4, 1024, 4096
N_CORES = 8
STRIDES = (1, 2, 4, 8)

FULL_CFG = dict(T=T, S=S, D=D_MODEL, hd=HEAD_DIM, strides=STRIDES, tt=512)


def build_program(cfg, debug=False):
    Tl, Sl, Dl, hd = cfg["T"], cfg["S"], cfg["D"], cfg["hd"]
    strides = cfg["strides"]
    ncls = len(strides)
    HP = 2 * hd * ncls  # packed head rows (8 heads * 64 = 512)

    nc = bacc.Bacc(
        "TRN2",
        target_bir_lowering=False,
        debug=False,
        enable_asserts=False,
        num_devices=N_CORES,
    )

    xT = nc.dram_tensor("xT", [Dl, Tl], BF, kind="ExternalInput").ap()
    eTs = {
        s: nc.dram_tensor(f"eT{s}", [Dl, Sl // s], BF, kind="ExternalInput").ap()
        for s in strides
    }
    wqT = nc.dram_tensor("wqT", [Dl, HP], BF, kind="ExternalInput").ap()
    wkT = nc.dram_tensor("wkT", [Dl, HP], BF, kind="ExternalInput").ap()
    wvT = nc.dram_tensor("wvT", [Dl, HP], BF, kind="ExternalInput").ap()
    woT = nc.dram_tensor("woT", [HP, Dl], BF, kind="ExternalInput").ap()
    # per-partition biases: cols [bq pairs 0..ncls-1 | bk pairs] (bq pre-scaled)
    bqk = nc.dram_tensor("bqk", [128, 2 * ncls], F32, kind="ExternalInput").ap()
    # V bias per (class, head): [128, hd] broadcast tiles, col (2c+h)*hd
    bvb = nc.dram_tensor("bvb", [128, ncls * 2 * hd], F32, kind="ExternalInput").ap()
    out = nc.dram_tensor("partial", [Tl, Dl], BF, kind="ExternalOutput").ap()

    dbg = None
    if debug:
        nck0 = (Sl + 127) // 128
        dbg = {
            "qt": [
                nc.dram_tensor(f"dbg_qt{c}", [128, Tl], BF, kind="ExternalOutput").ap()
                for c in range(ncls)
            ],
            "kt0": nc.dram_tensor("dbg_kt0", [128, Sl], BF, kind="ExternalOutput").ap(),
            "vt0": nc.dram_tensor(
                "dbg_vt0", [128, nck0 * (2 * hd + 2)], BF, kind="ExternalOutput"
            ).ap(),
            "den": [
                nc.dram_tensor(f"dbg_den{h}", [1, Tl], F32, kind="ExternalOutput").ap()
                for h in range(2)
            ],
            "aa0": nc.dram_tensor("dbg_aa0", [128, Tl], F32, kind="ExternalOutput").ap(),
            "ot": [
                nc.dram_tensor(f"dbg_ot{c}", [128, Tl], BF, kind="ExternalOutput").ap()
                for c in range(ncls)
            ],
        }

    with tile.TileContext(nc) as tc:
        _build(tc, cfg, xT, eTs, wqT, wkT, wvT, woT, bqk, bvb, out, dbg)

    nc.compile()
    return nc


def _build(tc, cfg, xT, eTs, wqT, wkT, wvT, woT, bqk, bvb, out, dbg=None):
    nc = tc.nc
    Tl, Sl, Dl, hd = cfg["T"], cfg["S"], cfg["D"], cfg["hd"]
    strides, tt = cfg["strides"], cfg["tt"]
    ncls = len(strides)
    HP = 2 * hd * ncls
    ndc = Dl // 128
    vw = 2 * hd + 2  # per-chunk V tile width: [V_A | ones | V_B | ones]
    t0s = list(range(0, Tl, tt))
    tws = [min(tt, Tl - t0) for t0 in t0s]
    ntt = len(t0s)

    with ExitStack() as ctx:
        wpool = ctx.enter_context(tc.tile_pool(name="w", bufs=1))
        qpool = ctx.enter_context(tc.tile_pool(name="qt", bufs=1))
        kpool = ctx.enter_context(tc.tile_pool(name="kt", bufs=1))
        vpool = ctx.enter_context(tc.tile_pool(name="vt", bufs=1))
        ptpool = ctx.enter_context(tc.tile_pool(name="pt", bufs=3))
        otpool = ctx.enter_context(tc.tile_pool(name="ot", bufs=1))
        npool = ctx.enter_context(tc.tile_pool(name="nrm", bufs=1))
        obpool = ctx.enter_context(tc.tile_pool(name="ob", bufs=2))
        a_ps = ctx.enter_context(tc.tile_pool(name="a_ps", bufs=2, space="PSUM"))
        # scores: one 2-bank tile per chunk so exp runs as a single [rw, 2*tt]
        # ACT instruction (halves the per-instruction overhead)
        sc_ps = ctx.enter_context(tc.tile_pool(name="sc_ps", bufs=2, space="PSUM"))
        av_ps = ctx.enter_context(tc.tile_pool(name="av_ps", bufs=1, space="PSUM"))

        # ---- weights / biases / x into SBUF ----
        # issue order matters: Q-proj needs wq+xt, then the class-0 K/V
        # projections need wk/wv + eT1 (the eT1 DMAs are issued by
        # dma_class_et(0) right after these); wo is only needed at the end
        bqk_sb = wpool.tile([128, 2 * ncls], F32, tag="bqk", name="bqk")
        nc.sync.dma_start(out=bqk_sb, in_=bqk)
        wq_sb = [wpool.tile([128, HP], BF, tag=f"wq{dc}", name="wq") for dc in range(ndc)]
        xt_sb = [wpool.tile([128, Tl], BF, tag=f"xt{dc}", name="xt") for dc in range(ndc)]
        for dc in range(ndc):
            nc.sync.dma_start(out=wq_sb[dc], in_=wqT[dc * 128 : (dc + 1) * 128, :])
        # xT lands in T-halves so the first (nt-major) Q groups start sooner
        for t0, tw in zip(t0s, tws):
            for dc in range(ndc):
                nc.sync.dma_start(
                    out=xt_sb[dc][:, t0 : t0 + tw],
                    in_=xT[dc * 128 : (dc + 1) * 128, t0 : t0 + tw],
                )
        # wk/wv/bvb are issued AFTER the class-0 encoder tiles (dma_class_et
        # below) — Q-proj covers ~14us of compute, and K/V-proj needs the
        # encoder data anyway, so eT1 gets the DMA bandwidth first
        wk_sb = [wpool.tile([128, HP], BF, tag=f"wk{dc}", name="wk") for dc in range(ndc)]
        wv_sb = [wpool.tile([128, HP], BF, tag=f"wv{dc}", name="wv") for dc in range(ndc)]
        bvb_sb = wpool.tile([128, ncls * 2 * hd], F32, tag="bvb", name="bvb")

        def dma_kv_weights():
            for dc in range(ndc):
                nc.sync.dma_start(out=wk_sb[dc], in_=wkT[dc * 128 : (dc + 1) * 128, :])
                nc.sync.dma_start(out=wv_sb[dc], in_=wvT[dc * 128 : (dc + 1) * 128, :])
            nc.sync.dma_start(out=bvb_sb, in_=bvb)
        # wo + out-staging pools are created late (once the per-class eT
        # pools are gone) so their space comes out of the freed eT region —
        # the up-front persistent pools otherwise don't leave room for eT0
        late = {}

        def make_late_pools():
            # on the RIGHT side: their lifetime overlaps several eT pools,
            # which live on the default (left) stack
            wop = ctx.enter_context(tc.tile_pool(name="wo", bufs=1, side="right"))
            late["wo"] = [wop.tile([128, Dl], BF, tag=f"wo{c}", name="wo") for c in range(ncls)]
            for c in range(ncls):
                nc.sync.dma_start(out=late["wo"][c], in_=woT[c * 128 : (c + 1) * 128, :])
            late["ob1p"] = ctx.enter_context(tc.tile_pool(name="ob1", bufs=1, side="right"))
            late["obp"] = ctx.enter_context(tc.tile_pool(name="ob", bufs=4, side="right"))

        # persistent per-class SBUF tiles
        qt = [qpool.tile([128, Tl], BF, tag=f"q{c}", name="qt") for c in range(ncls)]
        kt, vt, nck_c = {}, {}, {}
        for c in range(ncls):
            Ns = Sl // strides[c]
            nck_c[c] = (Ns + 127) // 128
            kt[c] = kpool.tile([128, Ns], BF, tag=f"k{c}", name="kt")
            vt[c] = vpool.tile([128, nck_c[c] * vw], BF, tag=f"v{c}", name="vt")
            # ones columns at hd and 2*hd+1 within each vw-wide chunk slot
            nc.gpsimd.memset(vt[c][:, hd :: hd + 1], 1.0)
        ot = [otpool.tile([128, Tl], BF, tag=f"o{c}", name="ot") for c in range(ncls)]
        avacc = [npool.tile([hd, Tl], F32, tag=f"aa{h}", name="avacc") for h in range(2)]
        den = [npool.tile([1, Tl], F32, tag=f"dn{h}", name="den") for h in range(2)]

        # encoder tiles per class; pool entered/exited at specific emission
        # points so only one class's eT is resident at a time
        et_sb = {}
        et_cm = {}

        def dma_class_et(c, mid=None):
            # alternate SBUF sides so consecutive classes' pools can overlap
            # in time while each side's pool stack stays LIFO
            s = strides[c]
            Ns = Sl // s
            cm = tc.tile_pool(name=f"et{s}", bufs=1)
            pool = cm.__enter__()
            et_cm[c] = cm
            tiles = [pool.tile([128, Ns], BF, tag=f"e{dc}", name="et") for dc in range(ndc)]
            et_sb[c] = tiles
            piece = max(512, ((Ns // 4 + 511) // 512) * 512)
            for c0 in range(0, Ns, piece):
                cw = min(piece, Ns - c0)
                for dc in range(ndc):
                    nc.sync.dma_start(
                        out=tiles[dc][:, c0 : c0 + cw],
                        in_=eTs[s][dc * 128 : (dc + 1) * 128, c0 : c0 + cw],
                    )
                if mid is not None:
                    mid()
                    mid = None

        def close_class_et(c):
            et_cm.pop(c).__exit__(None, None, None)
            et_sb.pop(c)

        # ---------- phase-A thunks (each emits one PE psum group) ----------
        def q_thunk(c, nt):
            def th():
                t0, tw = t0s[nt], tws[nt]
                ps = a_ps.tile([128, tt], F32, tag="a", name="a_ps")
                for dc in range(ndc):
                    nc.tensor.matmul(
                        ps[:, :tw],
                        wq_sb[dc][:, c * 128 : (c + 1) * 128],
                        xt_sb[dc][:, t0 : t0 + tw],
                        start=(dc == 0),
                        stop=(dc == ndc - 1),
                    )
                nc.vector.tensor_scalar_add(
                    qt[c][:, t0 : t0 + tw], ps[:, :tw], bqk_sb[:, c : c + 1]
                )

            return th

        def k_thunk(c, c0, cw):
            def th():
                ps = a_ps.tile([128, tt], F32, tag="a", name="a_ps")
                for dc in range(ndc):
                    nc.tensor.matmul(
                        ps[:, :cw],
                        wk_sb[dc][:, c * 128 : (c + 1) * 128],
                        et_sb[c][dc][:, c0 : c0 + cw],
                        start=(dc == 0),
                        stop=(dc == ndc - 1),
                    )
                nc.vector.tensor_scalar_add(
                    kt[c][:, c0 : c0 + cw], ps[:, :cw], bqk_sb[:, ncls + c : ncls + c + 1]
                )

            return th

        def v_thunk(c, g):
            # group g covers up to 2 S-chunks of 128 rows each; one PSUM
            # tile per chunk (start=True may clear the whole bank on HW).
            # Small groups interleave more evenly with the attention chunks.
            def th():
                Ns = Sl // strides[c]
                for ck in range(2 * g, min(2 * g + 2, nck_c[c])):
                    rw = min(128, Ns - ck * 128)
                    ps = a_ps.tile([128, tt], F32, tag="a", name="a_ps")
                    for dc in range(ndc):
                        nc.tensor.matmul(
                            ps[:rw, : 2 * hd],
                            et_sb[c][dc][:, ck * 128 : ck * 128 + rw],
                            wv_sb[dc][:, c * 128 : (c + 1) * 128],
                            start=(dc == 0),
                            stop=(dc == ndc - 1),
                        )
                    for h in range(2):
                        nc.vector.tensor_add(
                            vt[c][:rw, ck * vw + h * (hd + 1) : ck * vw + h * (hd + 1) + hd],
                            ps[:rw, h * hd : (h + 1) * hd],
                            bvb_sb[:rw, (2 * c + h) * hd : (2 * c + h + 1) * hd],
                        )

            return th

        def class_a_thunks(c):
            """K/V projection thunks as (needed_by_chunk, thunk): thunk i
            covers S-chunks 4i..4i+3 and must be emitted before the class's
            attention reads chunk 4i."""
            Ns = Sl // strides[c]
            merged = [
                (4 * i, th) for i, th in enumerate(
                    k_thunk(c, c0, min(tt, Ns - c0)) for c0 in range(0, Ns, tt)
                )
            ] + [(2 * g, v_thunk(c, g)) for g in range((nck_c[c] + 1) // 2)]
            merged.sort(key=lambda x: x[0])
            return merged

        # ---------- phase B for one class (attention) ----------
        def b_class(c, filler, pre=None, after_h0=None, pre_next=None):
            # pre: this class's own K/V projection thunks, emitted paced
            # (a few chunks ahead of the attention that consumes them) so
            # the class starts as soon as its first encoder pieces land.
            # pre_next: the NEXT class's first K/V thunks, emitted over the
            # second half of head 1 (head 1 is otherwise ACT-bound and the
            # next encoder tiles have landed by then).
            Ns = Sl // strides[c]
            nck = nck_c[c]
            fill_i, emitted = 0.0, 0
            fill_step = len(filler) / max(1.0, 0.7 * 2 * nck)
            pre = list(pre) if pre else []
            pre_next = list(pre_next) if pre_next else []
            pn_total, pn_emitted = len(pre_next), 0
            LA = 2  # lookahead chunks for pre-thunks

            for h in range(2):
                hb = h * hd
                avp = [
                    av_ps.tile([hd + 1, tws[nt]], F32, tag=f"av{nt}", name="av_ps")
                    for nt in range(ntt)
                ]
                pend = None  # software pipeline: AV lags scores/exp by 1 chunk
                for ck in range(nck):
                    if h == 0:
                        while pre and pre[0][0] <= ck + LA:
                            pre.pop(0)[1]()
                    elif pre_next:
                        # spread over the second half of head 1
                        start = nck - nck // 2
                        if ck >= start:
                            want = (ck - start + 1) * pn_total // max(1, nck - start)
                            while pn_emitted < want and pre_next:
                                pre_next.pop(0)[1]()
                                pn_emitted += 1
                    rw = min(128, Ns - ck * 128)
                    sps = sc_ps.tile([128, ntt * tt], F32, tag="sc", name="sc_ps")
                    for nt in range(ntt):
                        nc.tensor.matmul(
                            sps[:rw, nt * tt : nt * tt + tws[nt]],
                            kt[c][hb : hb + hd, ck * 128 : ck * 128 + rw],
                            qt[c][hb : hb + hd, t0s[nt] : t0s[nt] + tws[nt]],
                            start=True,
                            stop=True,
                        )
                    pt = ptpool.tile([128, ntt * tt], BF, tag="pt", name="pt")
                    if ntt * tt == Tl:
                        nc.scalar.activation(pt[:rw, :], sps[:rw, :], AF.Exp)
                    else:
                        for nt in range(ntt):
                            nc.scalar.activation(
                                pt[:rw, nt * tt : nt * tt + tws[nt]],
                                sps[:rw, nt * tt : nt * tt + tws[nt]],
                                AF.Exp,
                            )
                    if pend is not None:
                        pck, prw, ppt = pend
                        for nt in range(ntt):
                            nc.tensor.matmul(
                                avp[nt],
                                vt[c][:prw, pck * vw + h * (hd + 1) : pck * vw + h * (hd + 1) + hd + 1],
                                ppt[:prw, nt * tt : nt * tt + tws[nt]],
                                start=(pck == 0),
                                stop=False,
                            )
                    pend = (ck, rw, pt)
                    fill_i += fill_step
                    while emitted < min(int(fill_i), len(filler)):
                        filler[emitted]()
                        emitted += 1
                pck, prw, ppt = pend
                for nt in range(ntt):
                    nc.tensor.matmul(
                        avp[nt],
                        vt[c][:prw, pck * vw + h * (hd + 1) : pck * vw + h * (hd + 1) + hd + 1],
                        ppt[:prw, nt * tt : nt * tt + tws[nt]],
                        start=(pck == 0),
                        stop=True,
                    )
                # drain AV psum to SBUF fast (frees the banks for the next
                # head), then normalize this head right away — head A's
                # normalize hides under head B's attention
                for nt in range(ntt):
                    t0, tw = t0s[nt], tws[nt]
                    nc.vector.tensor_copy(avacc[h][:, t0 : t0 + tw], avp[nt][:hd, :])
                    nc.vector.tensor_copy(
                        den[h][0:1, t0 : t0 + tw], avp[nt][hd : hd + 1, :]
                    )
                normalize_head(c, h)
                if h == 0 and after_h0 is not None:
                    # all encoder reads (pre-thunks) are done — the caller
                    # closes this class's eT pool and starts the next DMA
                    after_h0()
            while emitted < len(filler):
                filler[emitted]()
                emitted += 1
            while pre_next:
                pre_next.pop(0)[1]()

        def normalize_head(c, h):
            # NB: partition_broadcast writes garbage when the output AP does
            # not start at partition 0 — use a [hd, T] tile per head.
            r = npool.tile([1, Tl], F32, tag=f"r{h}", name="recip")
            nc.vector.reciprocal_approx_fast(r, den[h])
            rb = npool.tile([hd, Tl], F32, tag=f"rb{h}", name="rb")
            nc.gpsimd.partition_broadcast(rb, r)
            nc.vector.tensor_mul(ot[c][h * hd : (h + 1) * hd, :], avacc[h], rb)

        # ---------- emission ----------
        # all Q projections run up front: they only need wq/xT (the first
        # DMAs), keeping the PE busy (and the HAM clock warming) while the
        # class-0 encoder tiles stream in
        dma_class_et(0, mid=dma_kv_weights)
        for nt in range(ntt):
            for cq in range(ncls):
                q_thunk(cq, nt)()

        # out projection in two stages: stage 1 (first half of the classes)
        # runs as filler inside the last attention class; stage 2 finishes
        # after the final normalize
        ob1 = {}

        def oproj_thunk(tc_i, d0, cls, first):
            def th():
                dw = min(tt, Dl - d0)
                ops = a_ps.tile([128, tt], F32, tag="a", name="a_ps")
                for i, c in enumerate(cls):
                    nc.tensor.matmul(
                        ops[:, :dw],
                        ot[c][:, tc_i * 128 : (tc_i + 1) * 128],
                        late["wo"][c][:, d0 : d0 + dw],
                        start=(i == 0),
                        stop=(i == len(cls) - 1),
                    )
                if first:
                    ob = late["ob1p"].tile([128, tt], BF, tag=f"ob1_{tc_i}_{d0}", name="ob1")
                    nc.vector.tensor_copy(ob[:, :dw], ops[:, :dw])
                    ob1[(tc_i, d0)] = ob
                else:
                    ob = late["obp"].tile([128, tt], BF, tag="ob", name="ob")
                    nc.vector.tensor_add(ob[:, :dw], ops[:, :dw], ob1[(tc_i, d0)][:, :dw])
                    nc.sync.dma_start(
                        out=out[tc_i * 128 : (tc_i + 1) * 128, d0 : d0 + dw],
                        in_=ob[:, :dw],
                    )

            return th

        o_groups = [(tc_i, d0) for tc_i in range(Tl // 128) for d0 in range(0, Dl, tt)]
        ncls1 = max(1, ncls // 2)  # classes in out-proj stage 1
        stage1 = [oproj_thunk(tc_i, d0, list(range(ncls1)), True) for tc_i, d0 in o_groups]

        pres = {}
        for c in range(ncls):
            filler = []
            if c == ncls - 2:
                # stage-1 out-proj (classes 0..ncls1-1): ot ready by now
                make_late_pools()
                filler.extend(stage1[: len(stage1) // 2])
            elif c == ncls - 1:
                # rest of stage 1; a few groups held back to keep the PE
                # busy while the final normalize runs
                filler.extend(stage1[len(stage1) // 2 : -4])

            def after_h0(c=c):
                # head 0 consumed all of this class's encoder tiles: free
                # the pool and queue the next class's encoder DMA (it lands
                # while head 1 runs)
                close_class_et(c)
                if c + 1 < ncls:
                    dma_class_et(c + 1)

            pre = pres.pop(c, None) or class_a_thunks(c)
            pre_next = None
            if c + 1 < ncls:
                nxt = class_a_thunks(c + 1)
                half = len(nxt) // 2
                pre_next, pres[c + 1] = nxt[:half], nxt[half:]
            b_class(c, filler, pre=pre, after_h0=after_h0, pre_next=pre_next)
            if dbg is not None and c == 0:
                nc.sync.dma_start(out=dbg["kt0"], in_=kt[0])
                nc.sync.dma_start(out=dbg["vt0"], in_=vt[0])
                for h in range(2):
                    nc.sync.dma_start(out=dbg["den"][h], in_=den[h])
                    nc.sync.dma_start(out=dbg["aa0"][h * hd : (h + 1) * hd, :], in_=avacc[h])
            if c == ncls - 1:
                # PE filler while the (DVE/GpSimd) normalize tail runs
                for th in stage1[-4:]:
                    th()
            if dbg is not None:
                nc.sync.dma_start(out=dbg["ot"][c], in_=ot[c])
                if c == ncls - 1:
                    for cq in range(ncls):
                        nc.sync.dma_start(out=dbg["qt"][cq], in_=qt[cq])

        for tc_i, d0 in o_groups:
            oproj_thunk(tc_i, d0, list(range(ncls1, ncls)), False)()


# ---------------------------------------------------------------------------
# Host-side sharding / gathering
# ---------------------------------------------------------------------------


def _core_heads(core, ncls=4):
    """[(class, headA, headB)] for this core; both heads share stride."""
    j = core % 2
    return [(ci, ci + 8 * j, ci + 8 * j + 4) for ci in range(ncls)]


def shard_inputs(inputs, cfg):
    x = np.asarray(inputs["decoder_input"], np.float32)
    e = np.asarray(inputs["encoder_output"], np.float32)
    Wq = np.asarray(inputs["Wq"], np.float32)
    Wk = np.asarray(inputs["Wk"], np.float32)
    Wv = np.asarray(inputs["Wv"], np.float32)
    Wo = np.asarray(inputs["Wo"], np.float32)
    bq = np.asarray(inputs["bq"], np.float32)
    bk = np.asarray(inputs["bk"], np.float32)
    bv = np.asarray(inputs["bv"], np.float32)
    hd = cfg["hd"]
    strides = cfg["strides"]
    ncls = len(strides)
    scale = 1.0 / np.sqrt(hd)
    in_maps = []
    for core in range(N_CORES):
        b = core // 2
        rows = []
        for ci, hA, hB in _core_heads(core, ncls):
            rows += list(range(hA * hd, (hA + 1) * hd)) + list(range(hB * hd, (hB + 1) * hd))
        rows = np.array(rows)
        m = {
            "xT": np.ascontiguousarray(x[b].T.astype(BF16)),
            "wqT": np.ascontiguousarray((Wq[rows] * scale).T.astype(BF16)),
            "wkT": np.ascontiguousarray(Wk[rows].T.astype(BF16)),
            "wvT": np.ascontiguousarray(Wv[rows].T.astype(BF16)),
            "woT": np.ascontiguousarray(Wo[:, rows].T.astype(BF16)),
        }
        for s in strides:
            m[f"eT{s}"] = np.ascontiguousarray(e[b, ::s].T.astype(BF16))
        bqk = np.zeros((128, 2 * ncls), np.float32)
        bvb = np.zeros((128, ncls * 2 * hd), np.float32)
        for ci, hA, hB in _core_heads(core, ncls):
            bqk[:, ci] = (
                np.concatenate([bq[hA * hd : (hA + 1) * hd], bq[hB * hd : (hB + 1) * hd]])
                * scale
            )
            bqk[:, ncls + ci] = np.concatenate(
                [bk[hA * hd : (hA + 1) * hd], bk[hB * hd : (hB + 1) * hd]]
            )
            bvb[:, (2 * ci) * hd : (2 * ci + 1) * hd] = bv[hA * hd : (hA + 1) * hd][None, :]
            bvb[:, (2 * ci + 1) * hd : (2 * ci + 2) * hd] = bv[hB * hd : (hB + 1) * hd][None, :]
        m["bqk"] = bqk
        m["bvb"] = bvb
        in_maps.append(m)
    return in_maps


def gather_output(results, bo, cfg):
    Tl, Dl = cfg["T"], cfg["D"]
    out = np.zeros((B, Tl, Dl), np.float32)
    for core in range(N_CORES):
        out[core // 2] += np.asarray(results[core]["partial"], np.float32)
    return out + np.asarray(bo, np.float32)[None, None, :]


_COMPILED = None


def _get_compiled():
    global _COMPILED
    if _COMPILED is None:
        _COMPILED = build_program(FULL_CFG)
    return _COMPILED


def run_on_cores(inputs, trace=False, **kw):
    nc = _get_compiled()
    in_maps = shard_inputs(inputs, FULL_CFG)
    res = bass_utils.run_bass_kernel_spmd(
        nc, in_maps, core_ids=list(range(N_CORES)), trace=trace, **kw
    )
    return res


def kernel(**inputs) -> np.ndarray:
    res = run_on_cores(inputs, trace=False)
    return gather_output(res.results, inputs["bo"], FULL_CFG)
